# revision 42
# baseline (speedup 1.0000x reference)
"""Trainium2 Bass kernel for nn_DynamicsLookAheadModel.

LSTM warm-up over S=96 steps + 32-step look-ahead with output feedback,
data-parallel over the batch (2048) across 8 NeuronCores (256 per core).

v2 design (vs v1 baseline at 1.043 ms):
  - "Light" warm-up steps 0..K_LIGHT-1 run the gate matmuls with a
    2-product bf16 scheme (w exact via hi+lo, h rounded to bf16): gate
    noise ~2^-9 injected before step 80 decays through the forget gate
    for >=16 steps before any STE bits are emitted (CPU-sim rel err
    0.0082 vs 0.0097 for the all-heavy scheme; budget 2e-2).
    Steps K_LIGHT..95 and all LA steps use the exact 3-product scheme.
  - Single-step processing with PSUM ping-pong: each step uses 4 banks
    (bank = gate, the two 128-dim halves on bank columns 0:256/256:512),
    sets alternate between steps, and the next step's x-projection
    matmuls are issued right after this step's h-matmuls so the PE fills
    the o-ACT -> h-cast dependency window.
  - LA phase: the static x rows (6:32, zero-padded to K=32) are
    precomputed bf16 hi/lo and open the PSUM banks with no feedback
    dependency; the h-matmuls accumulate; the K=6 output-feedback
    product (osb split to bf16 hi/lo on-chip) closes the banks last, so
    the po -> ACT -> split -> replicate chain hides under the h stream.
    W_fc runs as bf16 hi/lo (bits are exact 0/1 in bf16).
"""

from contextlib import ExitStack

import numpy as np

import concourse.bass as bass
import concourse.mybir as mybir
import concourse.tile as tile
from concourse.bass_utils import run_bass_kernel_spmd

B, S, F, H, O = 2048, 96, 32, 256, 6
LA = 32
NCORES = 8
BL = B // NCORES  # 256 per-core batch
K_LIGHT = 80  # warmup steps 0..K_LIGHT-1 use the 2-product scheme
FP32 = mybir.dt.float32
BF16 = mybir.dt.bfloat16


# --- workaround: this walrus build allows only ONE sem wait per instruction ---
def _spill_excess_waits(nc, limit=1):
    cnt = 0
    for f in nc.m.functions:
        for bb in f.blocks:
            new_list = []
            for ins in bb.instructions:
                si = ins.sync_info
                if si and si.on_wait and len(si.on_wait) > limit:
                    waits = list(si.on_wait)
                    for w in waits[:-limit]:
                        n = mybir.InstNoOp(name=f"wspill_{cnt}", ins=[], outs=[])
                        cnt += 1
                        n.engine = ins.engine
                        n.sync_info = mybir.SyncInfo(on_wait=[w], on_update=[])
                        new_list.append(n)
                    ins.sync_info = mybir.SyncInfo(
                        on_wait=waits[-limit:], on_update=list(si.on_update)
                    )
                new_list.append(ins)
            bb.instructions[:] = new_list
    return cnt


def build_nc(n_warm=S, n_la=LA, k_light=K_LIGHT, spill=True, dump_state=False):
    nc = bass.Bass()
    AF = mybir.ActivationFunctionType
    ALU = mybir.AluOpType
    if dump_state:
        dbgc_d = nc.dram_tensor("dbgc", [2, 128, BL], FP32, kind="ExternalOutput")
        dbgh_d = nc.dram_tensor("dbgh", [2, 128, BL], BF16, kind="ExternalOutput")

    k_light = min(k_light, n_warm)
    n_heavy = n_warm - k_light
    # light warmup x: bf16 hi only, [128, 256] per step (4-band replicated)
    xlt_d = nc.dram_tensor("xlt", [k_light, 128, BL], BF16, kind="ExternalInput")
    # heavy warmup x: bf16 hi|lo on columns, [128, 512] per step
    if n_heavy:
        xhv_d = nc.dram_tensor(
            "xhv", [n_heavy, 128, 2 * BL], BF16, kind="ExternalInput"
        )
    # LA static x: bf16 hi|lo, rows 32b:32b+6 of each band zeroed on host
    if n_la:
        xla_d = nc.dram_tensor("xla", [n_la, 128, 2 * BL], BF16, kind="ExternalInput")
    # W_ih.T [32, 1024] bf16 hi/lo, replicated over the 4 row bands
    wrh_d = nc.dram_tensor("wrh", [128, 4 * H], BF16, kind="ExternalInput")
    wrl_d = nc.dram_tensor("wrl", [128, 4 * H], BF16, kind="ExternalInput")
    # W_hh.T in split bf16 (hi + residual lo)
    whh0h_d = nc.dram_tensor("whh0h", [128, 4 * H], BF16, kind="ExternalInput")
    whh0l_d = nc.dram_tensor("whh0l", [128, 4 * H], BF16, kind="ExternalInput")
    whh1h_d = nc.dram_tensor("whh1h", [128, 4 * H], BF16, kind="ExternalInput")
    whh1l_d = nc.dram_tensor("whh1l", [128, 4 * H], BF16, kind="ExternalInput")
    # W_fc folded [128, 12] bf16 hi/lo
    wfch_d = nc.dram_tensor("wfch", [128, 2 * O], BF16, kind="ExternalInput")
    wfcl_d = nc.dram_tensor("wfcl", [128, 2 * O], BF16, kind="ExternalInput")
    bias8_d = nc.dram_tensor("bias8", [128, 8], FP32, kind="ExternalInput")
    # LA bias: bias + W_ih[:, :6] @ b_fc folded in (the ofb matmul consumes
    # the raw PSUM po, which excludes b_fc)
    bias8la_d = nc.dram_tensor("bias8la", [128, 8], FP32, kind="ExternalInput")
    bfc_d = nc.dram_tensor("bfc", [O, 1], FP32, kind="ExternalInput")
    out_d = nc.dram_tensor("out_t", [n_la + 1, O, BL], FP32, kind="ExternalOutput")

    with tile.TileContext(nc) as tc, ExitStack() as es:
        wp_ctx = es.enter_context(tc.tile_pool(name="weights", bufs=1))
        xp_ctx = es.enter_context(tc.tile_pool(name="xtiles", bufs=1))
        sp_ctx = es.enter_context(tc.tile_pool(name="state", bufs=2))
        gp_ctx = es.enter_context(tc.tile_pool(name="gates", bufs=1, space="PSUM"))

        # first light x tiles before the weight bulk so step-0 isn't queued
        # behind it
        xlt = []
        for t in range(min(3, k_light)):
            xt = xp_ctx.tile([128, BL], BF16, tag=f"xl{t}")
            nc.sync.dma_start(out=xt, in_=xlt_d[t, :, :])
            xlt.append(xt)
        wrh = wp_ctx.tile([128, 4 * H], BF16, tag="wrh")
        nc.sync.dma_start(out=wrh, in_=wrh_d[:, :])
        wrl = wp_ctx.tile([128, 4 * H], BF16, tag="wrl")
        nc.sync.dma_start(out=wrl, in_=wrl_d[:, :])
        whh0h = wp_ctx.tile([128, 4 * H], BF16, tag="whh0h")
        nc.sync.dma_start(out=whh0h, in_=whh0h_d[:, :])
        whh1h = wp_ctx.tile([128, 4 * H], BF16, tag="whh1h")
        nc.sync.dma_start(out=whh1h, in_=whh1h_d[:, :])
        whh0l = wp_ctx.tile([128, 4 * H], BF16, tag="whh0l")
        nc.sync.dma_start(out=whh0l, in_=whh0l_d[:, :])
        whh1l = wp_ctx.tile([128, 4 * H], BF16, tag="whh1l")
        nc.sync.dma_start(out=whh1l, in_=whh1l_d[:, :])
        bias8 = wp_ctx.tile([128, 8], FP32, tag="bias8")
        nc.sync.dma_start(out=bias8, in_=bias8_d[:, :])
        bias8la = wp_ctx.tile([128, 8], FP32, tag="bias8la")
        nc.sync.dma_start(out=bias8la, in_=bias8la_d[:, :])
        wfch = wp_ctx.tile([128, 2 * O], BF16, tag="wfch")
        nc.sync.dma_start(out=wfch, in_=wfch_d[:, :])
        wfcl = wp_ctx.tile([128, 2 * O], BF16, tag="wfcl")
        nc.sync.dma_start(out=wfcl, in_=wfcl_d[:, :])
        bfc = wp_ctx.tile([O, 1], FP32, tag="bfc")
        nc.sync.dma_start(out=bfc, in_=bfc_d[:, :])
        xhv = []
        for t in range(min(4, n_heavy)):
            xt = xp_ctx.tile([128, 2 * BL], BF16, tag=f"xv{t}")
            nc.sync.dma_start(out=xt, in_=xhv_d[t, :, :])
            xhv.append(xt)
        for t in range(min(3, k_light), k_light):
            xt = xp_ctx.tile([128, BL], BF16, tag=f"xl{t}")
            nc.sync.dma_start(out=xt, in_=xlt_d[t, :, :])
            xlt.append(xt)
        for t in range(min(4, n_heavy), n_heavy):
            xt = xp_ctx.tile([128, 2 * BL], BF16, tag=f"xv{t}")
            nc.sync.dma_start(out=xt, in_=xhv_d[t, :, :])
            xhv.append(xt)
        xlat = []
        for k in range(n_la):
            xt = xp_ctx.tile([128, 2 * BL], BF16, tag=f"xa{k}")
            nc.sync.dma_start(out=xt, in_=xla_d[k, :, :])
            xlat.append(xt)

        whh = [(whh0h, whh0l), (whh1h, whh1l)]
        GATE_FUNC = [AF.Sigmoid, AF.Sigmoid, AF.Tanh, AF.Sigmoid]  # i, f, g, o
        # h-matmul emission order (gate, half): g first (c-path), o last
        EMIT = [(2, 0), (2, 1), (1, 0), (1, 1), (0, 0), (0, 1), (3, 0), (3, 1)]

        def alloc_set(s):
            return [
                gp_ctx.tile(
                    [128, 2 * BL], FP32, tag=f"pb{s}{g}", name=f"pb{s}{g}"
                )
                for g in range(4)
            ]

        def x_open(banks, xt, prods, first=False, with_bias=False):
            """x-projection matmuls; open the accumulation regions.

            prods: list of (w_tile, x_col_off). Each (j, half) round issues
            4 concurrent band matmuls into the 4 distinct gate banks.
            with_bias: prepend K=32 zero-padded bias matmuls (light steps:
            bias lands in PSUM so the gate ACTs can merge both halves).
            """
            if with_bias:
                for r in (0, 1):
                    for b in range(4):
                        if first and b == 1:
                            continue
                        m = 2 * b + r
                        nc.tensor.matmul(
                            banks[b][:, BL * r : BL * r + BL],
                            biasmm[32 * b : 32 * b + 32, 128 * m : 128 * m + 128],
                            ones[32 * b : 32 * b + 32, :],
                            start=(r == 0),  # one bank-clear per bank
                            stop=False,
                            tile_position=(32 * b, 0),
                            skip_group_check=True,
                        )
            for j, (w_t, xoff) in enumerate(prods):
                for r in (0, 1):
                    for b in range(4):
                        if first and b == 1:
                            continue  # f gate unused at step 0
                        m = 2 * b + r
                        nc.tensor.matmul(
                            banks[b][:, BL * r : BL * r + BL],
                            w_t[32 * b : 32 * b + 32, 128 * m : 128 * m + 128],
                            xt[32 * b : 32 * b + 32, xoff : xoff + BL],
                            # ONE start per bank: the start flag clears
                            # has_written for the WHOLE bank, so region r=1
                            # must not re-start (it would wipe r=0's data);
                            # its first write overwrites via cleared bits.
                            start=(not with_bias and j == 0 and r == 0),
                            stop=False,
                            tile_position=(32 * b, 0),
                            skip_group_check=True,
                        )

        def region_mms(g, r, banks, prods, close_j=None):
            m = 2 * g + r
            dst = banks[g][:, BL * r : BL * r + BL]
            col = 128 * m
            for j, (w_t, h_t) in enumerate(prods):
                nc.tensor.matmul(
                    dst,
                    w_t[:, col : col + 128],
                    h_t[:, :],
                    start=False,
                    stop=(j == close_j),
                    skip_group_check=True,
                )

        def h_mms(banks, h_prev, nprod, close=True, mid_cb=None):
            """hh products per region in EMIT order; hl products (nprod=3)
            deferred 3 regions so the producing tail's DVE keeps up; mid_cb
            (LA feedback) is emitted after the hh products, before any close.
            """
            hh_prods = []
            for k in (0, 1):
                wh, wl = whh[k]
                hh_prods += [(wh, h_prev[k][0]), (wl, h_prev[k][0])][: min(nprod, 2)]
            hl_prods = (
                [(whh[k][0], h_prev[k][1]) for k in (0, 1)] if nprod == 3 else []
            )
            if not hl_prods:
                for (g, r) in EMIT:
                    region_mms(g, r, banks, hh_prods,
                               close_j=len(hh_prods) - 1 if close else None)
                if mid_cb is not None:
                    mid_cb()
                return
            if mid_cb is None:
                # warmup heavy: interleave, deferring each region's hl
                # products by 3 regions so closes stay staggered
                DEFER = 3
                for idx, (g, r) in enumerate(EMIT):
                    region_mms(g, r, banks, hh_prods)
                    if idx >= DEFER:
                        g2, r2 = EMIT[idx - DEFER]
                        region_mms(g2, r2, banks, hl_prods,
                                   close_j=1 if close else None)
                for idx in range(len(EMIT) - DEFER, len(EMIT)):
                    g2, r2 = EMIT[idx]
                    region_mms(g2, r2, banks, hl_prods,
                               close_j=1 if close else None)
            else:
                # LA: all hh products, then the feedback (fed is ready by
                # then), then hl products close each region
                for (g, r) in EMIT:
                    region_mms(g, r, banks, hh_prods)
                mid_cb()
                for (g, r) in EMIT:
                    region_mms(g, r, banks, hl_prods,
                               close_j=1 if close else None)

        def ofb_mms(banks, fed):
            """K=6 output-feedback products (mid-accumulation, no close)."""
            prods = [(wrh, 0), (wrl, 0), (wrh, BL)]
            for j, (w_t, xoff) in enumerate(prods):
                for r in (0, 1):
                    for b in range(4):
                        m = 2 * b + r
                        nc.tensor.matmul(
                            banks[b][:, BL * r : BL * r + BL],
                            w_t[32 * b : 32 * b + 6, 128 * m : 128 * m + 128],
                            fed[32 * b : 32 * b + 6, xoff : xoff + BL],
                            start=False,
                            stop=False,
                            tile_position=(32 * b, 0),
                            skip_group_check=True,
                        )

        def gate_acts_merged(banks, first=False):
            # light steps: bias is already in PSUM (biasmm matmul), so one
            # [128, 512] ACT per gate covers both halves; o included
            act = {}
            for g in (2, 1, 0, 3):
                if first and g == 1:
                    continue
                am = sp_ctx.tile([128, 2 * BL], FP32, tag=f"am{g}", name=f"am{g}")
                nc.scalar.activation(out=am, in_=banks[g][:, :], func=GATE_FUNC[g])
                for r in (0, 1):
                    act[(g, r)] = am[:, BL * r : BL * r + BL]
            return act

        def gate_acts(banks, first=False, bias_t=None):
            bias_t = bias_t if bias_t is not None else bias8
            act = {}
            for (g, r) in [(2, 0), (2, 1), (1, 0), (1, 1), (0, 0), (0, 1)]:
                if first and g == 1:
                    continue
                m = 2 * g + r
                ah = sp_ctx.tile([128, BL], FP32, tag=f"a{g}_{r}")
                act[(g, r)] = ah
                nc.scalar.activation(
                    out=ah,
                    in_=banks[g][:, BL * r : BL * r + BL],
                    func=GATE_FUNC[g],
                    bias=bias_t[:, m : m + 1],
                )
            return act

        def tail(banks, act, h_prev, c_prev, first=False, emit_lo=False,
                 want_bits=False, skip_h=False, bias_t=None):
            bias_t = bias_t if bias_t is not None else bias8
            """c update + h production. Returns (h_new, c_new, bits)."""
            c_new = []
            bits_new = []
            for r in (0, 1):
                cn = sp_ctx.tile([128, BL], FP32, tag=f"c{r}")
                if first:
                    nc.vector.tensor_tensor(
                        out=cn, in0=act[(0, r)], in1=act[(2, r)], op=ALU.mult
                    )
                else:
                    t1 = sp_ctx.tile([128, BL], FP32, tag=f"t1_{r}")
                    nc.vector.tensor_tensor(
                        out=t1, in0=act[(1, r)], in1=c_prev[r], op=ALU.mult
                    )
                    t2 = sp_ctx.tile([128, BL], FP32, tag=f"t2_{r}")
                    nc.vector.tensor_tensor(
                        out=t2, in0=act[(0, r)], in1=act[(2, r)], op=ALU.mult
                    )
                    nc.vector.tensor_tensor(out=cn, in0=t1, in1=t2, op=ALU.add)
                c_new.append(cn)
                if want_bits:
                    bt = sp_ctx.tile([128, BL], BF16, tag=f"bits{r}")
                    nc.vector.tensor_scalar(
                        out=bt, in0=cn, scalar1=0.0, scalar2=None, op0=ALU.is_gt
                    )
                    bits_new.append(bt)
            h_new = []
            if not skip_h:
                # o-ACT + tanh per half, then hh for both halves on DVE (the
                # next step's first operands); the lo-residuals go to GPSIMD
                acts_o = []
                for r in (0, 1):
                    cn = c_new[r]
                    tc_h = sp_ctx.tile([128, BL], FP32, tag=f"tc{r}")
                    nc.scalar.activation(out=tc_h, in_=cn, func=AF.Tanh)
                    if (3, r) in act:
                        ao = act[(3, r)]  # pre-computed merged sigma(o)
                    else:
                        m = 2 * 3 + r
                        ao = sp_ctx.tile([128, BL], FP32, tag=f"a3_{r}")
                        nc.scalar.activation(
                            out=ao,
                            in_=banks[3][:, BL * r : BL * r + BL],
                            func=GATE_FUNC[3],
                            bias=bias_t[:, m : m + 1],
                        )
                    acts_o.append((ao, tc_h))
                hhs = []
                for r in (0, 1):
                    ao, tc_h = acts_o[r]
                    hh = sp_ctx.tile([128, BL], BF16, tag=f"hh{r}")
                    nc.vector.tensor_tensor(out=hh, in0=ao, in1=tc_h, op=ALU.mult)
                    hhs.append(hh)
                for r in (0, 1):
                    if emit_lo:
                        ao, tc_h = acts_o[r]
                        hn = sp_ctx.tile([128, BL], FP32, tag=f"hn{r}")
                        nc.vector.tensor_tensor(
                            out=hn, in0=ao, in1=tc_h, op=ALU.mult
                        )
                        hl = sp_ctx.tile([128, BL], BF16, tag=f"hl{r}")
                        nc.vector.tensor_tensor(
                            out=hl, in0=hn, in1=hhs[r], op=ALU.subtract
                        )
                        h_new.append((hhs[r], hl))
                    else:
                        h_new.append((hhs[r], None))
            return h_new, c_new, bits_new

        def emit_output(k, bits_cur, set_cur):
            # bits = (c' > 0); equals STE(h) since sigmoid(o)>0, tanh sign-pres.
            # po reuses a finished set's i-gate bank (its ACT reads are the
            # earliest of that step, so the WAR is long satisfied).
            po = gp_ctx.tile([O, BL], FP32, tag=f"pb{set_cur}0")
            for j, w_t in enumerate((wfch, wfcl)):
                for r in (0, 1):
                    nc.tensor.matmul(
                        po,
                        w_t[:, O * r : O * r + O],
                        bits_cur[r][:, :],
                        start=(j == 0 and r == 0),
                        stop=(j == 1 and r == 1),
                        skip_group_check=True,
                    )
            osb = sp_ctx.tile([O, BL], FP32, tag="osb")
            nc.scalar.activation(out=osb, in_=po, func=AF.Identity, bias=bfc)
            nc.sync.dma_start(out=out_d[k, :, :], in_=osb)
            return po

        # ---- step 0 (light, no h part; x products close the regions) ----
        s = 0
        banks = alloc_set(s)
        # step 0: x products must close: emit with stop on last product
        prods0 = [(wrh, 0), (wrl, 0)]
        for j, (w_t, xoff) in enumerate(prods0):
            for r in (0, 1):
                for b in range(4):
                    if b == 1:
                        continue  # f gate unused at step 0
                    m = 2 * b + r
                    nc.tensor.matmul(
                        banks[b][:, BL * r : BL * r + BL],
                        w_t[32 * b : 32 * b + 32, 128 * m : 128 * m + 128],
                        xlt[0][32 * b : 32 * b + 32, 0:BL],
                        start=(j == 0 and r == 0),
                        stop=(j == len(prods0) - 1),
                        tile_position=(32 * b, 0),
                        skip_group_check=True,
                    )
        act = gate_acts(banks, first=True)
        if n_warm > 1:
            # pre-open step 1 banks
            nbanks = alloc_set(1)
            if 1 < k_light:
                x_open(nbanks, xlt[1], [(wrh, 0), (wrl, 0)])
            else:
                x_open(nbanks, xhv[0], [(wrh, 0), (wrl, 0), (wrh, BL)])
        h_prev, c_prev, _ = tail(banks, act, None, None, first=True)

        # ---- warmup steps 1..n_warm-1 ----
        for t in range(1, n_warm):
            light = t < k_light
            s = t % 2
            banks = nbanks
            h_mms(banks, h_prev, 2 if light else 3)
            act = gate_acts(banks)
            # pre-open next step's banks with its x products
            nt = t + 1
            nbanks = alloc_set((t + 1) % 2)
            if nt < n_warm:
                if nt < k_light:
                    x_open(nbanks, xlt[nt], [(wrh, 0), (wrl, 0)])
                else:
                    xt = xhv[nt - k_light]
                    x_open(nbanks, xt, [(wrh, 0), (wrl, 0), (wrh, BL)])
            elif n_la:
                # first LA step's static x
                x_open(nbanks, xlat[0], [(wrh, 0), (wrl, 0), (wrh, BL)])
            last_light = t == k_light - 1
            h_prev, c_prev, bits = tail(
                banks, act, h_prev, c_prev,
                emit_lo=(not light) or last_light,
                want_bits=(t == n_warm - 1),
            )

        if dump_state:
            for r in (0, 1):
                nc.sync.dma_start(out=dbgc_d[r, :, :], in_=c_prev[r])
                nc.sync.dma_start(out=dbgh_d[r, :, :], in_=h_prev[r][0])

        # ---- look-ahead steps ----
        # po for output 0 runs right after the warmup's last h stream
        po = emit_output(0, bits, (n_warm - 1) % 2) if n_la else None
        for k in range(n_la):
            banks = nbanks  # already opened with static-x products
            # split raw po (no b_fc: it is folded into bias8la) into bf16
            # hi/lo and replicate over the 4 bands
            fed = sp_ctx.tile([128, 2 * BL], BF16, tag="fed")
            nc.vector.tensor_copy(out=fed[0:O, 0:BL], in_=po)
            nc.vector.scalar_tensor_tensor(
                out=fed[0:O, BL : BL + BL],
                in0=fed[0:O, 0:BL],
                scalar=-1.0,
                in1=po,
                op0=ALU.mult,
                op1=ALU.add,
            )
            nc.scalar.copy(out=fed[32 : 32 + O, :], in_=fed[0:O, :])
            nc.vector.tensor_copy(out=fed[64 : 64 + O, :], in_=fed[0:O, :])
            nc.scalar.copy(out=fed[96 : 96 + O, :], in_=fed[0:O, :])
            # hh products run while the fed chain resolves; the feedback
            # lands mid-accumulation; hl products close each region
            h_mms(banks, h_prev, 3, close=True, mid_cb=lambda: ofb_mms(banks, fed))
            act = gate_acts(banks, bias_t=bias8la)
            h_prev, c_prev, bits = tail(
                banks, act, h_prev, c_prev,
                emit_lo=True, want_bits=True,
                skip_h=(k == n_la - 1),
                bias_t=bias8la,
            )
            # next output's po right after this step's h stream on the PE
            po = emit_output(k + 1, bits, (n_warm + k) % 2)
            # pre-open next LA step's banks with its static x
            if k + 1 < n_la:
                nbanks = alloc_set((n_warm + k + 1) % 2)
                x_open(nbanks, xlat[k + 1], [(wrh, 0), (wrl, 0), (wrh, BL)])

    if spill:
        _spill_excess_waits(nc)
    return nc


def _host_prep(x, W_ih, W_hh, b_ih, b_hh, W_fc, b_fc):
    """Build the 8 per-core input maps."""
    import ml_dtypes as mld

    x = np.asarray(x, dtype=np.float32)
    W_ih = np.asarray(W_ih, dtype=np.float32)
    W_hh = np.asarray(W_hh, dtype=np.float32)
    b_ih = np.asarray(b_ih, dtype=np.float32)
    b_hh = np.asarray(b_hh, dtype=np.float32)
    W_fc = np.asarray(W_fc, dtype=np.float32)
    b_fc = np.asarray(b_fc, dtype=np.float32)

    bias = (b_ih + b_hh).astype(np.float32)
    w32t = np.ascontiguousarray(W_ih.T).astype(np.float32)  # [32, 1024]
    w_hi = w32t.astype(mld.bfloat16)
    w_lo = (w32t - w_hi.astype(np.float32)).astype(mld.bfloat16)
    whh_t = np.ascontiguousarray(W_hh.T).astype(np.float32)  # [256, 1024]
    whh_hi = whh_t.astype(mld.bfloat16)
    whh_lo = (whh_t - whh_hi.astype(np.float32)).astype(mld.bfloat16)
    wfc_fold = np.concatenate([W_fc.T[:128], W_fc.T[128:]], axis=1)  # [128, 12]
    wfc_hi = wfc_fold.astype(mld.bfloat16)
    wfc_lo = (wfc_fold - wfc_hi.astype(np.float32)).astype(mld.bfloat16)
    shared = {
        "wrh": np.ascontiguousarray(np.tile(w_hi, (4, 1))),
        "wrl": np.ascontiguousarray(np.tile(w_lo, (4, 1))),
        "whh0h": np.ascontiguousarray(whh_hi[:128]),
        "whh0l": np.ascontiguousarray(whh_lo[:128]),
        "whh1h": np.ascontiguousarray(whh_hi[128:]),
        "whh1l": np.ascontiguousarray(whh_lo[128:]),
        "wfch": np.ascontiguousarray(wfc_hi),
        "wfcl": np.ascontiguousarray(wfc_lo),
        "bias8": np.ascontiguousarray(bias.reshape(8, 128).T).astype(np.float32),
        "bias8la": np.ascontiguousarray(
            (bias + W_ih[:, :O] @ b_fc).reshape(8, 128).T
        ).astype(np.float32),
        "bfc": np.ascontiguousarray(b_fc.reshape(O, 1)).astype(np.float32),
    }
    in_maps = []
    for c in range(NCORES):
        xc = x[c * BL : (c + 1) * BL]  # [BL, S, F]
        xT = np.ascontiguousarray(xc.transpose(1, 2, 0)).astype(np.float32)  # [S,F,BL]
        x_hi = xT.astype(mld.bfloat16)
        x_lo = (xT - x_hi.astype(np.float32)).astype(mld.bfloat16)
        # light steps: hi only, band-replicated -> [K_LIGHT, 128, BL]
        xlt = np.ascontiguousarray(np.tile(x_hi[:K_LIGHT], (1, 4, 1)))
        # heavy steps: hi|lo on columns -> [S-K_LIGHT, 128, 2*BL]
        xhv = np.concatenate([x_hi[K_LIGHT:], x_lo[K_LIGHT:]], axis=2)
        xhv = np.ascontiguousarray(np.tile(xhv, (1, 4, 1)))
        # LA static x: rows 0:6 zeroed (fed rows live in a separate tile)
        xla_hi = x_hi[:LA].copy()
        xla_lo = x_lo[:LA].copy()
        xla_hi[:, :O, :] = 0
        xla_lo[:, :O, :] = 0
        xla = np.concatenate([xla_hi, xla_lo], axis=2)
        xla = np.ascontiguousarray(np.tile(xla, (1, 4, 1)))
        in_maps.append({"xlt": xlt, "xhv": xhv, "xla": xla, **shared})
    return in_maps


_NC_CACHE = {}


def _get_nc():
    if "nc" not in _NC_CACHE:
        _NC_CACHE["nc"] = build_nc()
    return _NC_CACHE["nc"]


def run(inputs, trace=False):
    in_maps = _host_prep(**inputs)
    nc = _get_nc()
    res = run_bass_kernel_spmd(nc, in_maps, core_ids=list(range(NCORES)), trace=trace)
    outs = []
    for c in range(NCORES):
        o = res.results[c]["out_t"]  # [33, 6, BL]
        outs.append(np.ascontiguousarray(o.transpose(2, 0, 1)))  # [BL, 33, 6]
    full = np.concatenate(outs, axis=0).astype(np.float32)  # [B, 33, 6]
    return full, res


def kernel(**inputs):
    full, _ = run(inputs, trace=False)
    return full


if __name__ == "__main__":
    t = build_nc()
    print("built ok")
